# revision 4
# baseline (speedup 1.0000x reference)
"""Trainium2 Bass kernel: per-aspect windowed attention (sparse_attention).

Reference math:
    proj[a,b,s,f] = sum_h doc[b,s,h] aspProj[a,h,f]
    score[a,b,s]  = sum_{w,f} proj[a,b,s+w-2,f] E[a,f,w]      (zero-padded in s)
    attn          = softmax_s(score)
    rep[a,b,f]    = sum_s proj[a,b,s,f] attn[a,b,s]

Algebraic refactor used here (proj is never materialized):
    K[a,h,w]     = sum_f aspProj[a,h,f] E[a,f,w]              (tiny, host-side)
    score[a,b,s] = sum_{w,h} doc[b,s+w-2,h] K[a,h,w]
    probs        = exp(score)          (scores are tiny; no max needed)
    attn         = probs / rowsum(probs)
    wdoc[a,b,h]  = sum_s probs[a,b,s] doc[b,s,h]
    rep[a,b,f]   = (sum_h wdoc[a,b,h] aspProj[a,h,f]) / rowsum(probs)[a,b]

Sharding: data-parallel over batch, 8 batches per NeuronCore x 8 cores,
no collectives. Host pre-packs doc in bf16 twice (natural [s,h] tiles for
the s-contraction and transposed+padded [h,s] for the h-contraction) so the
device never transposes the big tensor.

Engine APs must start at partition 0/32/64/96, so all per-batch (8-row)
tiles live at partition base 0; (64, X) assemblies go through SBUF->SBUF
DMA, which has no partition-alignment constraint.
"""

import numpy as np
import ml_dtypes

import concourse.bass as bass
import concourse.bacc as bacc
import concourse.mybir as mybir
import concourse.tile as tile
from concourse.bass_utils import run_bass_kernel_spmd

B, S, H, A, W = 64, 1024, 128, 8, 5
PAD = (W - 1) // 2
NCORES = 8
BL = B // NCORES          # local batches per core
SP = S + 2 * PAD          # padded seq width of docT
NPAIR = BL * A            # 64 (batch, aspect) pairs per core
ST = S // 128             # seq tiles per batch

BF16 = mybir.dt.bfloat16
FP32 = mybir.dt.float32

TRACE = False             # test.py flips this to profile
LAST_RESULT = None

_NC_CACHE = None


def _build_nc():
    nc = bacc.Bacc(
        "TRN2", target_bir_lowering=False, debug=False, num_devices=NCORES
    )

    docN = nc.dram_tensor("docN", [BL, 128, ST, H], BF16, kind="ExternalInput")
    docT = nc.dram_tensor("docT", [BL, H, SP], BF16, kind="ExternalInput")
    kw = nc.dram_tensor("kw", [H, W, A], BF16, kind="ExternalInput")
    aproj = nc.dram_tensor("aproj", [H, A, H], BF16, kind="ExternalInput")
    ident = nc.dram_tensor("ident", [128, 128], BF16, kind="ExternalInput")
    attn_o = nc.dram_tensor("attn_out", [BL, A, S], FP32, kind="ExternalOutput")
    rep_o = nc.dram_tensor("rep_out", [BL, A, H], FP32, kind="ExternalOutput")

    Exp = mybir.ActivationFunctionType.Exp
    Copy = mybir.ActivationFunctionType.Copy

    with tile.TileContext(nc) as tc:
        with (
            tc.tile_pool(name="const", bufs=1) as constp,
            tc.tile_pool(name="doc", bufs=BL) as docp,
            tc.tile_pool(name="work", bufs=1) as workp,
            tc.tile_pool(name="perb", bufs=3) as perbp,
            tc.tile_pool(name="io", bufs=2) as iop,
            tc.tile_pool(
                name="ps_score", bufs=2, space=bass.MemorySpace.PSUM
            ) as ps_score,
            tc.tile_pool(
                name="ps_misc", bufs=2, space=bass.MemorySpace.PSUM
            ) as ps_misc,
        ):
            kw_sb = constp.tile([H, W, A], BF16, tag="kw")
            nc.sync.dma_start(kw_sb[:], kw.ap()[:])
            aproj_sb = constp.tile([H, A, H], BF16, tag="aproj")
            nc.sync.dma_start(aproj_sb[:], aproj.ap()[:])
            id_sb = constp.tile([128, 128], BF16, tag="ident")
            nc.sync.dma_start(id_sb[:], ident.ap()[:])

            docT_sb, docN_sb = [], []
            for b in range(BL):
                dT = docp.tile([H, SP], BF16, tag="docT")
                nc.sync.dma_start(dT[:], docT.ap()[b])
                dN = docp.tile([128, ST, H], BF16, tag="docN")
                nc.sync.dma_start(dN[:], docN.ap()[b])
                docT_sb.append(dT)
                docN_sb.append(dN)

            # rowsum bookkeeping: sums2[a, b] = rowsum of probs for pair
            # (b, a); rs2 = 1/sums2
            sums2 = workp.tile([32, 32], FP32, tag="sums2")
            rs2 = workp.tile([32, 32], FP32, tag="rs2")
            attn_ball = workp.tile([NPAIR, S], BF16, tag="attn_ball")
            attnT = workp.tile([128, ST, NPAIR], BF16, tag="attnT")
            wall = workp.tile([NPAIR, H], BF16, tag="wall")
            wallT = workp.tile([H, NPAIR], BF16, tag="wallT")

            # --- scores + exp + normalize, per local batch (rows = aspects)
            for b in range(BL):
                ps = ps_score.tile([A, S], FP32, tag="score")
                for half in range(2):
                    for w in range(W):
                        nc.tensor.matmul(
                            ps[:, half * 512 : (half + 1) * 512],
                            kw_sb[:, w, :],
                            docT_sb[b][:, half * 512 + w : half * 512 + w + 512],
                            start=(w == 0),
                            stop=(w == W - 1),
                        )
                probs = perbp.tile([A, S], FP32, tag="probs")
                # scores are tiny (|s| < ~0.1): exp without max-subtraction
                # is the same softmax; accum_out gives the row sums.
                nc.scalar.activation(
                    probs[:], ps[:, :], Exp,
                    accum_out=sums2[0:A, b : b + 1],
                )
                nc.vector.reciprocal(
                    rs2[0:A, b : b + 1], sums2[0:A, b : b + 1]
                )
                attn_f = perbp.tile([A, S], FP32, tag="attn_f")
                nc.scalar.activation(
                    attn_f[:], probs[:], Copy, scale=rs2[0:A, b : b + 1]
                )
                nc.sync.dma_start(attn_o.ap()[b], attn_f[:])
                attn_bb = perbp.tile([A, S], BF16, tag="attn_bb")
                nc.vector.tensor_scalar_mul(
                    attn_bb[:], probs[:], rs2[0:A, b : b + 1]
                )
                # partition-offset assembly must go through DMA
                nc.sync.dma_start(
                    attn_ball[b * A : (b + 1) * A, :], attn_bb[:]
                )

            # --- attn^T tiles [s_tile: 128, pair: 64]
            for t in range(ST):
                psT = ps_misc.tile([128, NPAIR], BF16, tag="misc")
                nc.tensor.transpose(
                    psT[:],
                    attn_ball[:, t * 128 : (t + 1) * 128],
                    id_sb[:NPAIR, :NPAIR],
                )
                nc.vector.tensor_copy(attnT[:, t, :], psT[:])

            # --- wdoc[a, h] = sum_s attn[a, s] doc[s, h], per local batch
            for b in range(BL):
                psW = ps_misc.tile([A, H], FP32, tag="misc")
                for t in range(ST):
                    nc.tensor.matmul(
                        psW[:],
                        attnT[:, t, b * A : (b + 1) * A],
                        docN_sb[b][:, t, :],
                        start=(t == 0),
                        stop=(t == ST - 1),
                    )
                wtmp = perbp.tile([A, H], BF16, tag="wtmp")
                nc.vector.tensor_copy(wtmp[:], psW[:])
                nc.sync.dma_start(wall[b * A : (b + 1) * A, :], wtmp[:])

            psWT = ps_misc.tile([H, NPAIR], BF16, tag="misc")
            nc.tensor.transpose(psWT[:], wall[:], id_sb[:NPAIR, :NPAIR])
            nc.vector.tensor_copy(wallT[:], psWT[:])

            # --- rep[b, f] per aspect = sum_h wdoc[b, h] aspProj[a, h, f]
            # (attnT is already softmax-normalized, so no rescale here)
            wallT_v = wallT[:].rearrange("h (b a) -> h b a", a=A)
            for a in range(A):
                psR = ps_misc.tile([BL, H], FP32, tag="misc")
                nc.tensor.matmul(psR[:], wallT_v[:, :, a], aproj_sb[:, a, :])
                repf = iop.tile([BL, H], FP32, tag="repf")
                nc.vector.tensor_copy(repf[:], psR[:])
                nc.sync.dma_start(rep_o.ap()[:, a, :], repf[:])

    nc.compile()
    return nc


def _get_nc():
    global _NC_CACHE
    if _NC_CACHE is None:
        _NC_CACHE = _build_nc()
    return _NC_CACHE


def kernel(batch_docIn, aspEmbed_weight, aspProj):
    global LAST_RESULT
    doc = np.asarray(batch_docIn, dtype=np.float32)
    aspE = np.asarray(aspEmbed_weight, dtype=np.float32)
    aP = np.asarray(aspProj, dtype=np.float32)

    E = aspE.reshape(A, H, W)
    K = np.einsum("ahx,axw->ahw", aP, E)  # (A, H, W)
    kw_pm = np.ascontiguousarray(K.transpose(1, 2, 0)).astype(
        ml_dtypes.bfloat16
    )  # (H, W, A)
    aproj_pm = np.ascontiguousarray(aP.transpose(1, 0, 2)).astype(
        ml_dtypes.bfloat16
    )  # (H, A, H)
    identity = np.eye(128, dtype=ml_dtypes.bfloat16)

    doc_bf = doc.astype(ml_dtypes.bfloat16)  # (B, S, H)
    in_maps = []
    for c in range(NCORES):
        dc = doc_bf[c * BL : (c + 1) * BL]  # (BL, S, H)
        docN = np.ascontiguousarray(
            dc.reshape(BL, ST, 128, H).transpose(0, 2, 1, 3)
        )  # (BL, 128, ST, H): [b, p, t, h] = doc[b, t*128+p, h]
        docT = np.zeros((BL, H, SP), dtype=ml_dtypes.bfloat16)
        docT[:, :, PAD : PAD + S] = dc.transpose(0, 2, 1)
        in_maps.append(
            {
                "docN": docN,
                "docT": docT,
                "kw": kw_pm,
                "aproj": aproj_pm,
                "ident": identity,
            }
        )

    nc = _get_nc()
    res = run_bass_kernel_spmd(
        nc, in_maps, core_ids=list(range(NCORES)), trace=TRACE
    )
    LAST_RESULT = res

    attn = np.empty((B, A, S), dtype=np.float32)
    rep = np.empty((B, A, H), dtype=np.float32)
    for c in range(NCORES):
        attn[c * BL : (c + 1) * BL] = res.results[c]["attn_out"]
        rep[c * BL : (c + 1) * BL] = res.results[c]["rep_out"]
    return attn, rep


# revision 12
# speedup vs baseline: 1.0398x; 1.0398x over previous
"""Trainium2 Bass kernel: per-aspect windowed attention (sparse_attention).

Reference math:
    proj[a,b,s,f] = sum_h doc[b,s,h] aspProj[a,h,f]
    score[a,b,s]  = sum_{w,f} proj[a,b,s+w-2,f] E[a,f,w]      (zero-padded in s)
    attn          = softmax_s(score)
    rep[a,b,f]    = sum_s proj[a,b,s,f] attn[a,b,s]

Algebraic refactor (proj never materialized):
    K[a,h,w]     = sum_f aspProj[a,h,f] E[a,f,w]              (tiny, host-side)
    score[a,b,s] = sum_{w,h} doc[b,s+w-2,h] K[a,h,w]
    attn         = exp(score) / rowsum                         (scores are tiny)
    rep[a,b,f]   = sum_h (sum_s attn[a,b,s] doc[b,s,h]) aspProj[a,h,f]

Sharding: data-parallel over batch, 8 batches per NeuronCore x 8 cores, no
collectives. Host pre-packs doc in bf16 twice: natural [s,h] tiles for the
s-contraction (wdoc) and transposed+padded [h,s] for the h-contraction
(scores). Raw bacc (no Tile framework) with hand-placed semaphores to avoid
the Tile preamble/sem-reset overhead (~18us) and to control DMA batching,
PE warmup, and engine assignment of DMA issues.

Engine plan:
  Sync   : input DMAs (5 batched), attn f32 outputs (8), rep output (1)
  Tensor : 8 warmup MMs (HAM un-throttle), 80 score MMs, 64 wdoc MMs, 8 rep
  Scalar : 8x exp (with accumulated rowsum), 8 wall-assembly DMAs (HWDGE)
  Vector : reciprocal + 2 scale muls per batch, all 32x32 stream-transposes
  GpSimd : 8 attn-assembly SBUF->SBUF DMAs (SWDGE), final semaphore clear
"""

import numpy as np
import ml_dtypes

import concourse.bass as bass
import concourse.bacc as bacc
import concourse.mybir as mybir
from concourse.bass_utils import run_bass_kernel_spmd

B, S, H, A, W = 64, 1024, 128, 8, 5
PAD = (W - 1) // 2
NCORES = 8
BL = B // NCORES          # local batches per core
SP = S + 2 * PAD          # padded seq width of docT
NPAIR = BL * A            # 64 (batch, aspect) pairs per core
ST = S // 128             # seq tiles per batch

BF16 = mybir.dt.bfloat16
FP32 = mybir.dt.float32

TRACE = False             # test.py flips this to profile
LAST_RESULT = None
N_WARMUP = 8              # N=512 warmup matmuls (~3.4us cold = HAM window)

_NC_CACHE = None


def _build_nc():
    nc = bacc.Bacc(
        "TRN2", target_bir_lowering=False, debug=False, num_devices=NCORES
    )
    Exp = mybir.ActivationFunctionType.Exp

    wt_d = nc.dram_tensor("wt", [128, 1192], BF16, kind="ExternalInput")
    dT_d = [
        nc.dram_tensor("docTA", [128, 4, SP], BF16, kind="ExternalInput"),
        nc.dram_tensor("docTB", [128, 4, SP], BF16, kind="ExternalInput"),
    ]
    dN_d = [
        nc.dram_tensor("docNA", [128, 4, ST, H], BF16, kind="ExternalInput"),
        nc.dram_tensor("docNB", [128, 4, ST, H], BF16, kind="ExternalInput"),
    ]
    attn_o = nc.dram_tensor("attn_out", [BL, A, S], FP32, kind="ExternalOutput")
    rep_o = nc.dram_tensor("rep_out", [BL, A, H], FP32, kind="ExternalOutput")

    # ---- SBUF ----
    wt = nc.alloc_sbuf_tensor("wt_sb", [128, 1192], BF16)
    dT = [nc.alloc_sbuf_tensor(f"dT{i}", [128, 4, SP], BF16) for i in range(2)]
    dN = [
        nc.alloc_sbuf_tensor(f"dN{i}", [128, 4, ST, H], BF16) for i in range(2)
    ]
    probs = [nc.alloc_sbuf_tensor(f"probs{i}", [A, S], FP32) for i in range(2)]
    attn_f = [nc.alloc_sbuf_tensor(f"attnf{i}", [A, S], FP32) for i in range(2)]
    attn_bb = [nc.alloc_sbuf_tensor(f"attnb{i}", [A, S], BF16) for i in range(2)]
    attn_ball = nc.alloc_sbuf_tensor("attn_ball", [NPAIR, S], BF16)
    attnT = nc.alloc_sbuf_tensor("attnT", [128, ST, NPAIR], BF16)
    sums2 = nc.alloc_sbuf_tensor("sums2", [A, BL], FP32)
    rs2 = nc.alloc_sbuf_tensor("rs2", [A, BL], FP32)
    zb = nc.alloc_sbuf_tensor("zb", [A, 1], FP32)
    wtmp = [nc.alloc_sbuf_tensor(f"wtmp{i}", [A, H], BF16) for i in range(2)]
    wall = nc.alloc_sbuf_tensor("wall", [NPAIR, H], BF16)
    wallT = nc.alloc_sbuf_tensor("wallT", [H, NPAIR], BF16)
    rep_all = nc.alloc_sbuf_tensor("rep_all", [BL, A, H], FP32)

    idv = wt.ap()[:, 0:128]
    kwv = wt.ap()[:, 128 : 128 + W * A].rearrange("h (w a) -> h w a", a=A)
    apv = wt.ap()[:, 168 : 168 + A * H].rearrange("h (a f) -> h a f", f=H)
    wallT_v = wallT.ap().rearrange("h (b a) -> h b a", a=A)

    # ---- PSUM: banks 0-1 score0, 2-3 score1, 4 warmup junk, 5/6 wdoc+rep
    ps_score = [
        nc.place_psum_tensor(f"ps_s{i}", [A, S], FP32, bank=2 * i)
        for i in range(2)
    ]
    ps_warm = nc.place_psum_tensor("ps_warm", [128, 512], FP32, bank=4)
    ps_w = [
        nc.place_psum_tensor(f"ps_w{i}", [A, H], FP32, bank=5 + i)
        for i in range(2)
    ]

    sems = {}
    for name in [
        "w", "dTA", "dTB", "dNA", "dNB", "z", "mm", "exp", "mulbf", "mulf",
        "asm", "oattn", "tt", "wd", "wcp", "wasm", "wts", "rep", "repc",
        "orep",
    ]:
        sems[name] = nc.alloc_semaphore(f"S_{name}")

    with nc.Block() as block:

        @block.sync
        def _(sync):
            sync.dma_start(wt.ap()[:], wt_d.ap()[:]).then_inc(sems["w"], 16)
            sync.dma_start(dT[0].ap()[:], dT_d[0].ap()[:]).then_inc(
                sems["dTA"], 16
            )
            sync.dma_start(dT[1].ap()[:], dT_d[1].ap()[:]).then_inc(
                sems["dTB"], 16
            )
            sync.dma_start(dN[0].ap()[:], dN_d[0].ap()[:]).then_inc(
                sems["dNA"], 16
            )
            sync.dma_start(dN[1].ap()[:], dN_d[1].ap()[:]).then_inc(
                sems["dNB"], 16
            )
            for b in range(BL):
                # wait for the previous DMA on the same semaphore so
                # completions are ordered (cumulative thresholds stay valid)
                sync.wait_ge(sems["mulbf"], b + 1)
                sync.wait_ge(sems["asm"], 16 * b)
                sync.dma_start(
                    attn_ball.ap()[b * A : (b + 1) * A, :], attn_bb[b % 2].ap()[:]
                ).then_inc(sems["asm"], 16)
                sync.wait_ge(sems["mulf"], b + 1)
                sync.wait_ge(sems["oattn"], 16 * b)
                sync.dma_start(attn_o.ap()[b], attn_f[b % 2].ap()[:]).then_inc(
                    sems["oattn"], 16
                )
            sync.wait_ge(sems["repc"], A)
            sync.dma_start(rep_o.ap()[:], rep_all.ap()[:]).then_inc(
                sems["orep"], 16
            )
            sync.wait_ge(sems["oattn"], 16 * BL)
            sync.wait_ge(sems["orep"], 16)

        @block.tensor
        def _(pe):
            te = nc.tensor
            pe.wait_ge(sems["w"], 16)
            for _ in range(N_WARMUP):
                te.matmul(ps_warm.ap()[:, :], idv, wt.ap()[:, 0:512])
            # ---- scores: 10 accumulating MMs per local batch
            pe.wait_ge(sems["dTA"], 16)
            for b in range(BL):
                if b == 4:
                    pe.wait_ge(sems["dTB"], 16)
                if b >= 2:
                    pe.wait_ge(sems["exp"], b - 1)
                for half in range(2):
                    for w in range(W):
                        mm = te.matmul(
                            ps_score[b % 2].ap()[:, half * 512 : half * 512 + 512],
                            kwv[:, w, :],
                            dT[b // 4].ap()[
                                :, b % 4, half * 512 + w : half * 512 + w + 512
                            ],
                            start=(w == 0),
                            stop=(w == W - 1),
                        )
                mm.then_inc(sems["mm"], 1)
            # ---- wdoc: 8 accumulating MMs per local batch
            pe.wait_ge(sems["dNA"], 16)
            pe.wait_ge(sems["tt"], 4)
            for b in range(BL):
                if b == 4:
                    pe.wait_ge(sems["dNB"], 16)
                    pe.wait_ge(sems["tt"], 8)
                if b >= 2:
                    pe.wait_ge(sems["wcp"], b - 1)
                for t in range(ST):
                    mm = te.matmul(
                        ps_w[b % 2].ap()[:, :],
                        attnT.ap()[:, t, b * A : (b + 1) * A],
                        dN[b // 4].ap()[:, b % 4, t, :],
                        start=(t == 0),
                        stop=(t == ST - 1),
                    )
                mm.then_inc(sems["wd"], 1)
            # ---- rep
            pe.wait_ge(sems["wts"], 8)
            pe.wait_ge(sems["wcp"], BL)
            for a in range(A):
                if a >= 2:
                    pe.wait_ge(sems["repc"], a - 1)
                te.matmul(
                    ps_w[a % 2].ap()[:, :], wallT_v[:, :, a], apv[:, a, :]
                ).then_inc(sems["rep"], 1)

        @block.scalar
        def _(act):
            sc = nc.scalar
            act.wait_ge(sems["z"], 1)
            for b in range(BL):
                act.wait_ge(sems["mm"], b + 1)
                if b >= 2:
                    act.wait_ge(sems["mulf"], b - 1)
                    act.wait_ge(sems["mulbf"], b - 1)
                sc.activation(
                    probs[b % 2].ap()[:],
                    ps_score[b % 2].ap()[:, :],
                    Exp,
                    bias=zb.ap()[:, :],
                    accum_out=sums2.ap()[:, b : b + 1],
                ).then_inc(sems["exp"], 1)
            # wall assembly DMAs (HWDGE on ACT queue)
            for b in range(BL):
                act.wait_ge(sems["wcp"], b + 1)
                act.wait_ge(sems["wasm"], 16 * b)
                act.dma_start(
                    wall.ap()[b * A : (b + 1) * A, :], wtmp[b % 2].ap()[:]
                ).then_inc(sems["wasm"], 16)

        @block.vector
        def _(dve):
            v = nc.vector

            def dve_block(b):
                dve.wait_ge(sems["exp"], b + 1)
                v.reciprocal(rs2.ap()[:, b : b + 1], sums2.ap()[:, b : b + 1])
                v.drain()
                if b >= 2:
                    dve.wait_ge(sems["asm"], 16 * (b - 1))
                v.tensor_scalar_mul(
                    attn_bb[b % 2].ap()[:],
                    probs[b % 2].ap()[:],
                    rs2.ap()[:, b : b + 1],
                ).then_inc(sems["mulbf"], 1)
                if b >= 2:
                    dve.wait_ge(sems["oattn"], 16 * (b - 1))
                v.tensor_scalar_mul(
                    attn_f[b % 2].ap()[:],
                    probs[b % 2].ap()[:],
                    rs2.ap()[:, b : b + 1],
                ).then_inc(sems["mulf"], 1)

            def attnT_quarter(pi, ci):
                in_ = attn_ball.ap()[pi * 32 : (pi + 1) * 32, :].rearrange(
                    "p (t k c) -> p t k c", k=4, c=32
                )[:, :, ci, :]
                out = attnT.ap()[ci * 32 : (ci + 1) * 32, :, pi * 32 : pi * 32 + 32]
                v.transpose(out, in_).then_inc(sems["tt"], 1)

            for b in range(6):
                dve_block(b)
            dve.wait_ge(sems["asm"], 16 * 4)
            for ci in range(4):
                attnT_quarter(0, ci)
            dve_block(6)
            dve_block(7)
            dve.wait_ge(sems["asm"], 16 * 8)
            for ci in range(4):
                attnT_quarter(1, ci)
            # wdoc psum -> bf16 wtmp
            for b in range(BL):
                dve.wait_ge(sems["wd"], b + 1)
                if b >= 2:
                    dve.wait_ge(sems["wasm"], 16 * (b - 1))
                v.tensor_copy(wtmp[b % 2].ap()[:], ps_w[b % 2].ap()[:, :]).then_inc(
                    sems["wcp"], 1
                )
            # wall -> wallT (32x32 stream transpose blocks)
            for pi in range(2):
                dve.wait_ge(sems["wasm"], 16 * 4 * (pi + 1))
                for ci in range(4):
                    v.transpose(
                        wallT.ap()[ci * 32 : (ci + 1) * 32, pi * 32 : pi * 32 + 32],
                        wall.ap()[pi * 32 : (pi + 1) * 32, ci * 32 : (ci + 1) * 32],
                    ).then_inc(sems["wts"], 1)
            for a in range(A):
                dve.wait_ge(sems["rep"], a + 1)
                v.tensor_copy(rep_all.ap()[:, a, :], ps_w[a % 2].ap()[:, :]).then_inc(
                    sems["repc"], 1
                )

        @block.gpsimd
        def _(gp):
            nc.gpsimd.memset(zb.ap()[:], 0.0).then_inc(sems["z"], 1)
            # direct waits on every DMA-completion sem so the range-clear
            # provably happens after the last in-flight increment
            gp.wait_ge(sems["w"], 16)
            gp.wait_ge(sems["dTA"], 16)
            gp.wait_ge(sems["dTB"], 16)
            gp.wait_ge(sems["dNA"], 16)
            gp.wait_ge(sems["dNB"], 16)
            gp.wait_ge(sems["asm"], 16 * BL)
            gp.wait_ge(sems["oattn"], 16 * BL)
            gp.wait_ge(sems["wasm"], 16 * BL)
            gp.wait_ge(sems["orep"], 16)

        # full barrier, then reset every kernel semaphore so the NEFF can be
        # re-executed from a clean state
        nc.all_engine_barrier()
        nums = sorted(s.num for s in sems.values())
        assert nums[-1] - nums[0] == len(nums) - 1, nums
        nc.gpsimd.sem_clear(range(nums[0], nums[-1] + 1))

    nc.compile()
    return nc


def _get_nc():
    global _NC_CACHE
    if _NC_CACHE is None:
        _NC_CACHE = _build_nc()
    return _NC_CACHE


def kernel(batch_docIn, aspEmbed_weight, aspProj):
    global LAST_RESULT
    doc = np.asarray(batch_docIn, dtype=np.float32)
    aspE = np.asarray(aspEmbed_weight, dtype=np.float32)
    aP = np.asarray(aspProj, dtype=np.float32)

    E = aspE.reshape(A, H, W)
    K = np.einsum("ahx,axw->ahw", aP, E)  # (A, H, W)
    wt = np.zeros((128, 1192), dtype=ml_dtypes.bfloat16)
    wt[:, 0:128] = np.eye(128)
    wt[:, 128 : 128 + W * A] = (
        K.transpose(1, 2, 0).reshape(H, W * A).astype(ml_dtypes.bfloat16)
    )
    wt[:, 168 : 168 + A * H] = (
        aP.transpose(1, 0, 2).reshape(H, A * H).astype(ml_dtypes.bfloat16)
    )

    doc_bf = doc.astype(ml_dtypes.bfloat16)  # (B, S, H)
    in_maps = []
    for c in range(NCORES):
        dc = doc_bf[c * BL : (c + 1) * BL]  # (BL, S, H)
        m = {"wt": wt}
        for i, half in enumerate(("A", "B")):
            dh = dc[i * 4 : (i + 1) * 4]
            # docN[p, b, t, h] = doc[b, t*128+p, h]
            m[f"docN{half}"] = np.ascontiguousarray(
                dh.reshape(4, ST, 128, H).transpose(2, 0, 1, 3)
            )
            dTc = np.zeros((128, 4, SP), dtype=ml_dtypes.bfloat16)
            dTc[:, :, PAD : PAD + S] = dh.transpose(2, 0, 1)
            m[f"docT{half}"] = dTc
        in_maps.append(m)

    nc = _get_nc()
    res = run_bass_kernel_spmd(
        nc, in_maps, core_ids=list(range(NCORES)), trace=TRACE
    )
    LAST_RESULT = res

    attn = np.empty((B, A, S), dtype=np.float32)
    rep = np.empty((B, A, H), dtype=np.float32)
    for c in range(NCORES):
        attn[c * BL : (c + 1) * BL] = res.results[c]["attn_out"]
        rep[c * BL : (c + 1) * BL] = res.results[c]["rep_out"]
    return attn, rep


# revision 16
# speedup vs baseline: 1.2053x; 1.1592x over previous
"""Trainium2 Bass kernel: per-aspect windowed attention (sparse_attention).

Reference math:
    proj[a,b,s,f] = sum_h doc[b,s,h] aspProj[a,h,f]
    score[a,b,s]  = sum_{w,f} proj[a,b,s+w-2,f] E[a,f,w]      (zero-padded in s)
    attn          = softmax_s(score)
    rep[a,b,f]    = sum_s proj[a,b,s,f] attn[a,b,s]

Algebraic refactor (proj never materialized):
    K[a,h,w]     = sum_f aspProj[a,h,f] E[a,f,w]              (tiny, host-side)
    score[a,b,s] = sum_{w,h} doc[b,s+w-2,h] K[a,h,w]
    attn         = exp(score) / rowsum                         (scores are tiny)
    rep[a,b,f]   = sum_h (sum_s attn[a,b,s] doc[b,s,h]) aspProj[a,h,f]

Sharding: data-parallel over batch, 8 batches per NeuronCore x 8 cores, no
collectives. Host pre-packs doc in bf16 twice: natural [s,h] tiles for the
s-contraction (wdoc) and transposed+padded [h,s] for the h-contraction
(scores). Raw bacc (no Tile framework) with hand-placed semaphores to avoid
the Tile preamble/sem-reset overhead (~18us) and to control DMA batching,
PE warmup, and engine assignment of DMA issues.

Engine plan:
  Sync   : input DMAs (5 batched), attn f32 outputs (8), rep output (1)
  Tensor : 8 warmup MMs (HAM un-throttle), 80 score MMs, 64 wdoc MMs, 8 rep
  Scalar : 8x exp (with accumulated rowsum), 8 wall-assembly DMAs (HWDGE)
  Vector : reciprocal + 2 scale muls per batch, all 32x32 stream-transposes
  GpSimd : 8 attn-assembly SBUF->SBUF DMAs (SWDGE), final semaphore clear
"""

import numpy as np
import ml_dtypes

import concourse.bass as bass
import concourse.bacc as bacc
import concourse.mybir as mybir
from concourse.bass_utils import run_bass_kernel_spmd

B, S, H, A, W = 64, 1024, 128, 8, 5
PAD = (W - 1) // 2
NCORES = 8
BL = B // NCORES          # local batches per core
SP = S + 2 * PAD          # padded seq width of docT
NPAIR = BL * A            # 64 (batch, aspect) pairs per core
ST = S // 128             # seq tiles per batch

BF16 = mybir.dt.bfloat16
FP32 = mybir.dt.float32

TRACE = False             # test.py flips this to profile
LAST_RESULT = None
N_WARMUP = 8              # N=512 warmup matmuls (~3.4us cold = HAM window)

_NC_CACHE = None


def _build_nc():
    nc = bacc.Bacc(
        "TRN2", target_bir_lowering=False, debug=False, num_devices=NCORES
    )
    Exp = mybir.ActivationFunctionType.Exp

    kwid_d = nc.dram_tensor("kwid", [128, 168], BF16, kind="ExternalInput")
    apw_d = nc.dram_tensor("apw", [128, 1024], BF16, kind="ExternalInput")
    dT_d = [
        nc.dram_tensor("docTA", [128, 4, SP], BF16, kind="ExternalInput"),
        nc.dram_tensor("docTB", [128, 4, SP], BF16, kind="ExternalInput"),
    ]
    dN_d = [
        nc.dram_tensor("docNA", [128, 4, ST, H], BF16, kind="ExternalInput"),
        nc.dram_tensor("docNB", [128, 4, ST, H], BF16, kind="ExternalInput"),
    ]
    attn_o = nc.dram_tensor("attn_out", [BL, A, S], FP32, kind="ExternalOutput")
    rep_o = nc.dram_tensor("rep_out", [BL, A, H], FP32, kind="ExternalOutput")

    # ---- SBUF ----
    wt = nc.alloc_sbuf_tensor("wt_sb", [128, 1192], BF16)
    dT = [nc.alloc_sbuf_tensor(f"dT{i}", [128, 4, SP], BF16) for i in range(2)]
    dN = [
        nc.alloc_sbuf_tensor(f"dN{i}", [128, 4, ST, H], BF16) for i in range(2)
    ]
    probs = [nc.alloc_sbuf_tensor(f"probs{i}", [A, S], FP32) for i in range(2)]
    attn_f = [nc.alloc_sbuf_tensor(f"attnf{i}", [A, S], FP32) for i in range(2)]
    attn_bb = [nc.alloc_sbuf_tensor(f"attnb{i}", [A, S], BF16) for i in range(2)]
    attn_ball = nc.alloc_sbuf_tensor("attn_ball", [NPAIR, S], BF16)
    attnT = nc.alloc_sbuf_tensor("attnT", [128, ST, NPAIR], BF16)
    sums2 = nc.alloc_sbuf_tensor("sums2", [A, BL], FP32)
    rs2 = nc.alloc_sbuf_tensor("rs2", [A, BL], FP32)
    zb = nc.alloc_sbuf_tensor("zb", [A, 1], FP32)
    wtmp = [nc.alloc_sbuf_tensor(f"wtmp{i}", [A, H], BF16) for i in range(BL)]
    junk = nc.alloc_sbuf_tensor("junk", [128, 512], BF16)
    wall = nc.alloc_sbuf_tensor("wall", [NPAIR, H], BF16)
    wallT = nc.alloc_sbuf_tensor("wallT", [H, NPAIR], BF16)
    rep_all = nc.alloc_sbuf_tensor("rep_all", [BL, A, H], FP32)

    idv = wt.ap()[:, 0:128]
    kwv = wt.ap()[:, 128 : 128 + W * A].rearrange("h (w a) -> h w a", a=A)
    apv = wt.ap()[:, 168 : 168 + A * H].rearrange("h (a f) -> h a f", f=H)
    wallT_v = wallT.ap().rearrange("h (b a) -> h b a", a=A)

    # ---- PSUM: banks 0-1 score0, 2-3 score1, 4 warmup junk, 5/6 wdoc+rep
    ps_score = [
        nc.place_psum_tensor(f"ps_s{i}", [A, S], FP32, bank=2 * i)
        for i in range(2)
    ]
    ps_warm = nc.place_psum_tensor("ps_warm", [128, 512], FP32, bank=4)
    ps_w = [
        nc.place_psum_tensor(f"ps_w{i}", [A, H], FP32, bank=5 + i)
        for i in range(3)
    ]

    sems = {}
    sem_names = (
        ["kwid", "apw", "dTA", "dTB", "dNA", "dNB", "z", "mm", "exp",
         "mulbf", "mulf", "tt", "wd", "wcp", "wts", "rep", "repcv",
         "repcs", "orep"]
        + [f"asm{b}" for b in (1, 2, 3, 5, 6, 7)]
        + [f"oattn{p}" for p in range(BL)]
        + [f"wasm{b}" for b in range(BL)]
        + ["j"]
    )
    for name in sem_names:
        sems[name] = nc.alloc_semaphore(f"S_{name}")

    # process order: b=0 / b=4 go last in each half so their bf16 attn rows
    # (partition bases 0 / 32 -- the only engine-writable bases) are written
    # straight into attn_ball, skipping the assembly DMA on the critical path
    B_SEQ = [1, 2, 3, 0, 5, 6, 7, 4]
    DIRECT = {0, 4}

    with nc.Block() as block:

        @block.sync
        def _(sync):
            sync.dma_start(wt.ap()[:, 0:168], kwid_d.ap()[:]).then_inc(
                sems["kwid"], 16
            )
            sync.dma_start(dT[0].ap()[:], dT_d[0].ap()[:]).then_inc(
                sems["dTA"], 16
            )
            sync.dma_start(dT[1].ap()[:], dT_d[1].ap()[:]).then_inc(
                sems["dTB"], 16
            )
            sync.dma_start(dN[0].ap()[:], dN_d[0].ap()[:]).then_inc(
                sems["dNA"], 16
            )
            sync.dma_start(dN[1].ap()[:], dN_d[1].ap()[:]).then_inc(
                sems["dNB"], 16
            )
            sync.dma_start(wt.ap()[:, 168:1192], apw_d.ap()[:]).then_inc(
                sems["apw"], 16
            )
            for p, b in enumerate(B_SEQ):
                if b not in DIRECT:
                    sync.wait_ge(sems["mulbf"], p + 1)
                    sync.dma_start(
                        attn_ball.ap()[b * A : (b + 1) * A, :],
                        attn_bb[p % 2].ap()[:],
                    ).then_inc(sems[f"asm{b}"], 16)
                sync.wait_ge(sems["mulf"], p + 1)
                sync.dma_start(attn_o.ap()[b], attn_f[p % 2].ap()[:]).then_inc(
                    sems[f"oattn{p}"], 16
                )
            # wall assembly, first half (second half issued by ScalarE)
            for b in range(4):
                sync.wait_ge(sems["wcp"], b + 1)
                sync.dma_start(
                    wall.ap()[b * A : (b + 1) * A, :], wtmp[b].ap()[:]
                ).then_inc(sems[f"wasm{b}"], 16)
            sync.wait_ge(sems["repcv"], 4)
            sync.wait_ge(sems["repcs"], 4)
            sync.dma_start(rep_o.ap()[:], rep_all.ap()[:]).then_inc(
                sems["orep"], 16
            )
            for p in range(BL):
                sync.wait_ge(sems[f"oattn{p}"], 16)
            sync.wait_ge(sems["orep"], 16)

        @block.tensor
        def _(pe):
            te = nc.tensor
            # warmup on a zeroed junk tile: no DMA dependency, so the HAM
            # un-throttles while input DMAs are still in flight
            pe.wait_ge(sems["j"], 1)
            for _ in range(N_WARMUP):
                te.matmul(ps_warm.ap()[:, :], junk.ap()[:, 0:128], junk.ap()[:])
            # ---- scores: 10 accumulating MMs per local batch
            pe.wait_ge(sems["kwid"], 16)
            pe.wait_ge(sems["dTA"], 16)
            for p, b in enumerate(B_SEQ):
                if p == 4:
                    pe.wait_ge(sems["dTB"], 16)
                if p >= 2:
                    pe.wait_ge(sems["exp"], p - 1)
                for half in range(2):
                    for w in range(W):
                        mm = te.matmul(
                            ps_score[p % 2].ap()[:, half * 512 : half * 512 + 512],
                            kwv[:, w, :],
                            dT[b // 4].ap()[
                                :, b % 4, half * 512 + w : half * 512 + w + 512
                            ],
                            start=(w == 0),
                            stop=(w == W - 1),
                        )
                mm.then_inc(sems["mm"], 1)
            # ---- wdoc: 8 accumulating MMs per local batch, 3-bank rotation
            pe.wait_ge(sems["dNA"], 16)
            pe.wait_ge(sems["tt"], 4)
            for b in range(BL):
                if b == 4:
                    pe.wait_ge(sems["dNB"], 16)
                    pe.wait_ge(sems["tt"], 8)
                if b >= 3:
                    pe.wait_ge(sems["wcp"], b - 2)
                for t in range(ST):
                    mm = te.matmul(
                        ps_w[b % 3].ap()[:, :],
                        attnT.ap()[:, t, b * A : (b + 1) * A],
                        dN[b // 4].ap()[:, b % 4, t, :],
                        start=(t == 0),
                        stop=(t == ST - 1),
                    )
                mm.then_inc(sems["wd"], 1)
            # ---- rep
            pe.wait_ge(sems["wts"], 8)
            pe.wait_ge(sems["wcp"], BL)
            pe.wait_ge(sems["apw"], 16)
            for a in range(A):
                if a >= 3:
                    prev = a - 3
                    if prev % 2 == 0:
                        pe.wait_ge(sems["repcv"], prev // 2 + 1)
                    else:
                        pe.wait_ge(sems["repcs"], prev // 2 + 1)
                te.matmul(
                    ps_w[a % 3].ap()[:, :], wallT_v[:, :, a], apv[:, a, :]
                ).then_inc(sems["rep"], 1)

        @block.scalar
        def _(act):
            sc = nc.scalar
            act.wait_ge(sems["z"], 1)
            for p, b in enumerate(B_SEQ):
                act.wait_ge(sems["mm"], p + 1)
                if p >= 2:
                    act.wait_ge(sems["mulf"], p - 1)
                    act.wait_ge(sems["mulbf"], p - 1)
                sc.activation(
                    probs[p % 2].ap()[:],
                    ps_score[p % 2].ap()[:, :],
                    Exp,
                    bias=zb.ap()[:, :],
                    accum_out=sums2.ap()[:, b : b + 1],
                ).then_inc(sems["exp"], 1)
            # wall assembly, second half (HWDGE on ACT queue)
            for b in range(4, BL):
                act.wait_ge(sems["wcp"], b + 1)
                act.dma_start(
                    wall.ap()[b * A : (b + 1) * A, :], wtmp[b].ap()[:]
                ).then_inc(sems[f"wasm{b}"], 16)
            # rep psum -> sbuf copies, odd aspects (even on DVE)
            for a in range(1, A, 2):
                act.wait_ge(sems["rep"], a + 1)
                sc.copy(rep_all.ap()[:, a, :], ps_w[a % 3].ap()[:, :]).then_inc(
                    sems["repcs"], 1
                )

        @block.vector
        def _(dve):
            v = nc.vector
            v.memset(junk.ap()[:], 0.0).then_inc(sems["j"], 1)
            last_bb_user = {0: None, 1: None}  # slot -> asm sem name

            def pos_block(p, b):
                dve.wait_ge(sems["exp"], p + 1)
                v.reciprocal(rs2.ap()[:, b : b + 1], sums2.ap()[:, b : b + 1])
                v.drain()
                if b in DIRECT:
                    v.tensor_scalar_mul(
                        attn_ball.ap()[b * A : (b + 1) * A, :],
                        probs[p % 2].ap()[:],
                        rs2.ap()[:, b : b + 1],
                    ).then_inc(sems["mulbf"], 1)
                else:
                    prev = last_bb_user[p % 2]
                    if prev is not None:
                        dve.wait_ge(sems[prev], 16)
                    v.tensor_scalar_mul(
                        attn_bb[p % 2].ap()[:],
                        probs[p % 2].ap()[:],
                        rs2.ap()[:, b : b + 1],
                    ).then_inc(sems["mulbf"], 1)
                    last_bb_user[p % 2] = f"asm{b}"
                if p >= 2:
                    dve.wait_ge(sems[f"oattn{p - 2}"], 16)
                v.tensor_scalar_mul(
                    attn_f[p % 2].ap()[:],
                    probs[p % 2].ap()[:],
                    rs2.ap()[:, b : b + 1],
                ).then_inc(sems["mulf"], 1)

            def attnT_quarter(pi, ci):
                in_ = attn_ball.ap()[pi * 32 : (pi + 1) * 32, :].rearrange(
                    "p (t k c) -> p t k c", k=4, c=32
                )[:, :, ci, :]
                out = attnT.ap()[ci * 32 : (ci + 1) * 32, :, pi * 32 : pi * 32 + 32]
                v.transpose(out, in_).then_inc(sems["tt"], 1)

            for p in range(4):
                pos_block(p, B_SEQ[p])
            for b in (1, 2, 3):
                dve.wait_ge(sems[f"asm{b}"], 16)
            dve.wait_ge(sems["mulbf"], 4)  # retire b=0 direct write
            for ci in range(4):
                attnT_quarter(0, ci)
            for p in range(4, 8):
                pos_block(p, B_SEQ[p])
            for b in (5, 6, 7):
                dve.wait_ge(sems[f"asm{b}"], 16)
            dve.wait_ge(sems["mulbf"], 8)  # retire b=4 direct write
            for ci in range(4):
                attnT_quarter(1, ci)
            # wdoc psum -> bf16 wtmp (one buffer per batch, no reuse waits)
            for b in range(BL):
                dve.wait_ge(sems["wd"], b + 1)
                v.tensor_copy(wtmp[b].ap()[:], ps_w[b % 3].ap()[:, :]).then_inc(
                    sems["wcp"], 1
                )
            # wall -> wallT (32x32 stream transpose blocks)
            for pi in range(2):
                for b in range(4 * pi, 4 * pi + 4):
                    dve.wait_ge(sems[f"wasm{b}"], 16)
                for ci in range(4):
                    v.transpose(
                        wallT.ap()[ci * 32 : (ci + 1) * 32, pi * 32 : pi * 32 + 32],
                        wall.ap()[pi * 32 : (pi + 1) * 32, ci * 32 : (ci + 1) * 32],
                    ).then_inc(sems["wts"], 1)
            # rep psum -> sbuf copies, even aspects
            for a in range(0, A, 2):
                dve.wait_ge(sems["rep"], a + 1)
                v.tensor_copy(rep_all.ap()[:, a, :], ps_w[a % 3].ap()[:, :]).then_inc(
                    sems["repcv"], 1
                )

        @block.gpsimd
        def _(gp):
            nc.gpsimd.memset(zb.ap()[:], 0.0).then_inc(sems["z"], 1)

    nc.compile()
    return nc


def _get_nc():
    global _NC_CACHE
    if _NC_CACHE is None:
        _NC_CACHE = _build_nc()
    return _NC_CACHE


def kernel(batch_docIn, aspEmbed_weight, aspProj):
    global LAST_RESULT
    doc = np.asarray(batch_docIn, dtype=np.float32)
    aspE = np.asarray(aspEmbed_weight, dtype=np.float32)
    aP = np.asarray(aspProj, dtype=np.float32)

    E = aspE.reshape(A, H, W)
    K = np.einsum("ahx,axw->ahw", aP, E)  # (A, H, W)
    kwid = np.zeros((128, 168), dtype=ml_dtypes.bfloat16)
    kwid[:, 0:128] = np.eye(128)
    kwid[:, 128 : 128 + W * A] = (
        K.transpose(1, 2, 0).reshape(H, W * A).astype(ml_dtypes.bfloat16)
    )
    apw = np.ascontiguousarray(
        aP.transpose(1, 0, 2).reshape(H, A * H)
    ).astype(ml_dtypes.bfloat16)

    doc_bf = doc.astype(ml_dtypes.bfloat16)  # (B, S, H)
    in_maps = []
    for c in range(NCORES):
        dc = doc_bf[c * BL : (c + 1) * BL]  # (BL, S, H)
        m = {"kwid": kwid, "apw": apw}
        for i, half in enumerate(("A", "B")):
            dh = dc[i * 4 : (i + 1) * 4]
            # docN[p, b, t, h] = doc[b, t*128+p, h]
            m[f"docN{half}"] = np.ascontiguousarray(
                dh.reshape(4, ST, 128, H).transpose(2, 0, 1, 3)
            )
            dTc = np.zeros((128, 4, SP), dtype=ml_dtypes.bfloat16)
            dTc[:, :, PAD : PAD + S] = dh.transpose(2, 0, 1)
            m[f"docT{half}"] = dTc
        in_maps.append(m)

    nc = _get_nc()
    res = run_bass_kernel_spmd(
        nc, in_maps, core_ids=list(range(NCORES)), trace=TRACE
    )
    LAST_RESULT = res

    attn = np.empty((B, A, S), dtype=np.float32)
    rep = np.empty((B, A, H), dtype=np.float32)
    for c in range(NCORES):
        attn[c * BL : (c + 1) * BL] = res.results[c]["attn_out"]
        rep[c * BL : (c + 1) * BL] = res.results[c]["rep_out"]
    return attn, rep


# revision 20
# speedup vs baseline: 1.6042x; 1.3309x over previous
"""Trainium2 Bass kernel: per-aspect windowed attention (sparse_attention).

Reference math:
    proj[a,b,s,f] = sum_h doc[b,s,h] aspProj[a,h,f]
    score[a,b,s]  = sum_{w,f} proj[a,b,s+w-2,f] E[a,f,w]      (zero-padded in s)
    attn          = softmax_s(score)
    rep[a,b,f]    = sum_s proj[a,b,s,f] attn[a,b,s]

Algebraic refactor (proj never materialized):
    K[a,h,w]     = sum_f aspProj[a,h,f] E[a,f,w]              (tiny, host-side)
    score[a,b,s] = sum_{w,h} doc[b,s+w-2,h] K[a,h,w]
    attn         = exp(score) / rowsum                         (scores are tiny)
    rep[a,b,f]   = sum_h (sum_s attn[a,b,s] doc[b,s,h]) aspProj[a,h,f]

Sharding: data-parallel over batch, 8 batches per NeuronCore x 8 cores, no
collectives. Host pre-packs doc in bf16 twice: natural [s,h] tiles for the
s-contraction (wdoc) and transposed+padded [h,s] for the h-contraction
(scores). Raw bacc (no Tile framework) with hand-placed semaphores.

Engine plan:
  Sync   : batched input DMAs, attn output (1), wall assembly lo, rep out
  Tensor : warmup MMs (HAM un-throttle), 80 score MMs, 64 wdoc MMs,
           hi-half attn transposes, wall transpose, 8 rep MMs
  Scalar : dummy exp (ACT table preload), 8x exp+rowsum, wall assembly hi,
           odd rep copies
  Vector : per-batch softmax scale (bf16), lo-half 32x32 stream transposes,
           psum->sbuf copies, even rep copies
  GpSimd : zero-fill memset only

attn is produced in bf16 (attn_out dram is bf16, upcast on host); scores
are tiny so exp(score) ~ 1 +- 0.2 and bf16 keeps ~3 decimal digits, well
inside the accuracy budget.
"""

import numpy as np
import ml_dtypes

import concourse.bass as bass
import concourse.bacc as bacc
import concourse.mybir as mybir
from concourse.bass_utils import run_bass_kernel_spmd

B, S, H, A, W = 64, 1024, 128, 8, 5
PAD = (W - 1) // 2
NCORES = 8
BL = B // NCORES          # local batches per core
SP = S + 2 * PAD          # padded seq width of docT
NPAIR = BL * A            # 64 (batch, aspect) pairs per core
ST = S // 128             # seq tiles per batch

BF16 = mybir.dt.bfloat16
FP32 = mybir.dt.float32

TRACE = False             # test.py flips this to profile
LAST_RESULT = None
N_WARMUP = 8              # N=512 warmup matmuls (~3.4us cold = HAM window)

# process order: b=0 / b=4 go last in each half so their bf16 attn rows
# (partition bases 0 / 32 -- the only engine-writable bases) are written
# straight into attn_ball, skipping the assembly DMA on the critical path
B_SEQ = [1, 2, 3, 0, 5, 6, 7, 4]
DIRECT = {0, 4}

_NC_CACHE = None


def _build_nc():
    nc = bacc.Bacc(
        "TRN2", target_bir_lowering=False, debug=False, num_devices=NCORES
    )
    Exp = mybir.ActivationFunctionType.Exp

    kwid_d = nc.dram_tensor("kwid", [128, 168], BF16, kind="ExternalInput")
    apw_d = nc.dram_tensor("apw", [128, 1024], BF16, kind="ExternalInput")
    # docT packed in PROCESS order (B_SEQ), docN packed in batch order
    dT_d = [
        nc.dram_tensor("docTA", [128, 4, SP], BF16, kind="ExternalInput"),
        nc.dram_tensor("docTB", [128, 4, SP], BF16, kind="ExternalInput"),
    ]
    dN_d = [
        nc.dram_tensor("docNA", [128, 4, ST, H], BF16, kind="ExternalInput"),
        nc.dram_tensor("docNB", [128, 4, ST, H], BF16, kind="ExternalInput"),
    ]
    attn_o = nc.dram_tensor("attn_out", [NPAIR, S], BF16, kind="ExternalOutput")
    rep_o = nc.dram_tensor("rep_out", [BL, A, H], FP32, kind="ExternalOutput")

    # ---- SBUF ----
    wt = nc.alloc_sbuf_tensor("wt_sb", [128, 1192], BF16)
    dT = [nc.alloc_sbuf_tensor(f"dT{i}", [128, 4, SP], BF16) for i in range(2)]
    dN = [
        nc.alloc_sbuf_tensor(f"dN{i}", [128, 4, ST, H], BF16) for i in range(2)
    ]
    probs = [nc.alloc_sbuf_tensor(f"probs{i}", [A, S], FP32) for i in range(2)]
    attn_bb = [nc.alloc_sbuf_tensor(f"attnb{i}", [A, S], BF16) for i in range(2)]
    attn_ball = nc.alloc_sbuf_tensor("attn_ball", [NPAIR, S], BF16)
    attnT = nc.alloc_sbuf_tensor("attnT", [128, ST, NPAIR], BF16)
    sums2 = nc.alloc_sbuf_tensor("sums2", [A, BL], FP32)
    rs2 = nc.alloc_sbuf_tensor("rs2", [A, BL], FP32)
    zb = nc.alloc_sbuf_tensor("zb", [A, 1], FP32)
    scr = nc.alloc_sbuf_tensor("scr", [A, 1], FP32)
    wtmp = [nc.alloc_sbuf_tensor(f"wtmp{i}", [A, H], BF16) for i in range(BL)]
    junk = nc.alloc_sbuf_tensor("junk", [128, 512], BF16)
    wall = nc.alloc_sbuf_tensor("wall", [NPAIR, H], BF16)
    wallT = nc.alloc_sbuf_tensor("wallT", [H, NPAIR], BF16)
    rep_all = nc.alloc_sbuf_tensor("rep_all", [BL, A, H], FP32)

    idv = wt.ap()[:, 0:128]
    kwv = wt.ap()[:, 128 : 128 + W * A].rearrange("h (w a) -> h w a", a=A)
    apv = wt.ap()[:, 168 : 168 + A * H].rearrange("h (a f) -> h a f", f=H)
    wallT_v = wallT.ap().rearrange("h (b a) -> h b a", a=A)

    # ---- PSUM: banks 0-1 score0, 2-3 score1, 4 warmup junk, 5-7 wdoc/rep
    # rotation. psT (hi attn transpose) and psWT (wall transpose) alias
    # banks 0/2 -- the score tiles are dead by the time they are written.
    ps_score = [
        nc.place_psum_tensor(f"ps_s{i}", [A, S], FP32, bank=2 * i)
        for i in range(2)
    ]
    ps_warm = nc.place_psum_tensor("ps_warm", [128, 512], FP32, bank=4)
    ps_w = [
        nc.place_psum_tensor(f"ps_w{i}", [A, H], FP32, bank=5 + i)
        for i in range(3)
    ]
    psT = nc.place_psum_tensor("psT", [128, ST, 32], BF16, bank=0)
    psWT = nc.place_psum_tensor("psWT", [128, NPAIR], BF16, bank=2)

    sems = {}
    sem_names = (
        ["kwid", "apw", "dTA0", "dTA1", "dTB", "dNA", "dNB", "z", "j",
         "mm", "exp", "mulbf", "tt", "tp", "ttc", "wd", "wcp", "wtp",
         "wtc", "rep", "repcv", "repcs", "oattn", "orep"]
        + [f"asm{b}" for b in (1, 2, 3, 5, 6, 7)]
        + [f"wasm{b}" for b in range(BL)]
    )
    for name in sem_names:
        sems[name] = nc.alloc_semaphore(f"S_{name}")

    with nc.Block() as block:

        @block.sync
        def _(sync):
            sync.dma_start(wt.ap()[:, 0:168], kwid_d.ap()[:]).then_inc(
                sems["kwid"], 16
            )
            sync.dma_start(
                dT[0].ap()[:, 0:1, :], dT_d[0].ap()[:, 0:1, :]
            ).then_inc(sems["dTA0"], 16)
            sync.dma_start(
                dT[0].ap()[:, 1:4, :], dT_d[0].ap()[:, 1:4, :]
            ).then_inc(sems["dTA1"], 16)
            sync.dma_start(dT[1].ap()[:], dT_d[1].ap()[:]).then_inc(
                sems["dTB"], 16
            )
            sync.dma_start(dN[0].ap()[:], dN_d[0].ap()[:]).then_inc(
                sems["dNA"], 16
            )
            sync.dma_start(dN[1].ap()[:], dN_d[1].ap()[:]).then_inc(
                sems["dNB"], 16
            )
            sync.dma_start(wt.ap()[:, 168:1192], apw_d.ap()[:]).then_inc(
                sems["apw"], 16
            )
            # assembly DMAs for the six non-direct batches
            for p, b in enumerate(B_SEQ):
                if b in DIRECT:
                    continue
                sync.wait_ge(sems["mulbf"], p + 1)
                sync.dma_start(
                    attn_ball.ap()[b * A : (b + 1) * A, :],
                    attn_bb[p % 2].ap()[:],
                ).then_inc(sems[f"asm{b}"], 16)
            # single attn output DMA (bf16; host upcasts)
            sync.wait_ge(sems["mulbf"], BL)
            for b in (1, 2, 3, 5, 6, 7):
                sync.wait_ge(sems[f"asm{b}"], 16)
            sync.dma_start(attn_o.ap()[:], attn_ball.ap()[:]).then_inc(
                sems["oattn"], 16
            )
            # wall assembly, first half (second half issued by ScalarE)
            for b in range(4):
                sync.wait_ge(sems["wcp"], b + 1)
                sync.dma_start(
                    wall.ap()[b * A : (b + 1) * A, :], wtmp[b].ap()[:]
                ).then_inc(sems[f"wasm{b}"], 16)
            sync.wait_ge(sems["repcv"], 4)
            sync.wait_ge(sems["repcs"], 4)
            sync.dma_start(rep_o.ap()[:], rep_all.ap()[:]).then_inc(
                sems["orep"], 16
            )
            sync.wait_ge(sems["oattn"], 16)
            sync.wait_ge(sems["orep"], 16)

        @block.tensor
        def _(pe):
            te = nc.tensor
            # warmup on a zeroed junk tile: no DMA dependency, so the HAM
            # un-throttles while input DMAs are still in flight
            pe.wait_ge(sems["j"], 1)
            for _ in range(N_WARMUP):
                te.matmul(ps_warm.ap()[:, :], junk.ap()[:, 0:128], junk.ap()[:])
            # ---- scores: 10 accumulating MMs per local batch
            pe.wait_ge(sems["kwid"], 16)
            for p, b in enumerate(B_SEQ):
                if p == 0:
                    pe.wait_ge(sems["dTA0"], 16)
                elif p == 1:
                    pe.wait_ge(sems["dTA1"], 16)
                elif p == 4:
                    pe.wait_ge(sems["dTB"], 16)
                if p >= 2:
                    pe.wait_ge(sems["exp"], p - 1)
                for w in range(W):
                    for half in range(2):
                        mm = te.matmul(
                            ps_score[p % 2].ap()[:, half * 512 : half * 512 + 512],
                            kwv[:, w, :],
                            dT[p // 4].ap()[
                                :, p % 4, half * 512 + w : half * 512 + w + 512
                            ],
                            start=(w == 0),
                            stop=(w == W - 1),
                        )
                mm.then_inc(sems["mm"], 1)
            # ---- wdoc lo (b=0..3), needs the DVE lo-half transposes
            pe.wait_ge(sems["dNA"], 16)
            pe.wait_ge(sems["tt"], 4)
            for b in range(4):
                if b >= 3:
                    pe.wait_ge(sems["wcp"], b - 2)
                for t in range(ST):
                    mm = te.matmul(
                        ps_w[b % 3].ap()[:, :],
                        attnT.ap()[:, t, b * A : (b + 1) * A],
                        dN[0].ap()[:, b, t, :],
                        start=(t == 0),
                        stop=(t == ST - 1),
                    )
                mm.then_inc(sems["wd"], 1)
            # ---- hi-half attn transposes on PE (the critical chain):
            # attn_ball[32:64] -> psT -> (DVE copy) -> attnT[:, :, 32:64]
            for b in (5, 6, 7):
                pe.wait_ge(sems[f"asm{b}"], 16)
            pe.wait_ge(sems["mulbf"], BL)  # b=4 direct write
            for t in range(ST):
                te.matmul(
                    psT.ap()[:, t, :],
                    attn_ball.ap()[32:64, t * 128 : (t + 1) * 128],
                    idv[32:64, 32:64],
                    is_transpose=True,
                ).then_inc(sems["tp"], 1)
            # ---- wdoc hi
            pe.wait_ge(sems["dNB"], 16)
            pe.wait_ge(sems["ttc"], 1)
            for b in range(4, BL):
                if b >= 3:
                    pe.wait_ge(sems["wcp"], b - 2)
                for t in range(ST):
                    mm = te.matmul(
                        ps_w[b % 3].ap()[:, :],
                        attnT.ap()[:, t, b * A : (b + 1) * A],
                        dN[1].ap()[:, b - 4, t, :],
                        start=(t == 0),
                        stop=(t == ST - 1),
                    )
                mm.then_inc(sems["wd"], 1)
            # ---- wall transpose on PE
            for b in range(BL):
                pe.wait_ge(sems[f"wasm{b}"], 16)
            te.matmul(
                psWT.ap()[:, :],
                wall.ap()[:],
                idv[0:NPAIR, 0:NPAIR],
                is_transpose=True,
            ).then_inc(sems["wtp"], 1)
            # ---- rep
            pe.wait_ge(sems["wtc"], 1)
            pe.wait_ge(sems["wcp"], BL)
            pe.wait_ge(sems["apw"], 16)
            for a in range(A):
                if a >= 3:
                    prev = a - 3
                    if prev % 2 == 0:
                        pe.wait_ge(sems["repcv"], prev // 2 + 1)
                    else:
                        pe.wait_ge(sems["repcs"], prev // 2 + 1)
                te.matmul(
                    ps_w[a % 3].ap()[:, :], wallT_v[:, :, a], apv[:, a, :]
                ).then_inc(sems["rep"], 1)

        @block.scalar
        def _(act):
            sc = nc.scalar
            act.wait_ge(sems["z"], 1)
            # dummy exp so the ACT table load happens before the scores land
            sc.activation(scr.ap()[:], zb.ap()[:], Exp, bias=zb.ap()[:, :])
            for p, b in enumerate(B_SEQ):
                act.wait_ge(sems["mm"], p + 1)
                if p >= 2:
                    act.wait_ge(sems["mulbf"], p - 1)
                sc.activation(
                    probs[p % 2].ap()[:],
                    ps_score[p % 2].ap()[:, :],
                    Exp,
                    bias=zb.ap()[:, :],
                    accum_out=sums2.ap()[:, b : b + 1],
                ).then_inc(sems["exp"], 1)
            # wall assembly, second half (HWDGE on ACT queue)
            for b in range(4, BL):
                act.wait_ge(sems["wcp"], b + 1)
                act.dma_start(
                    wall.ap()[b * A : (b + 1) * A, :], wtmp[b].ap()[:]
                ).then_inc(sems[f"wasm{b}"], 16)
            # rep psum -> sbuf copies, odd aspects (even on DVE)
            for a in range(1, A, 2):
                act.wait_ge(sems["rep"], a + 1)
                sc.copy(rep_all.ap()[:, a, :], ps_w[a % 3].ap()[:, :]).then_inc(
                    sems["repcs"], 1
                )

        @block.vector
        def _(dve):
            v = nc.vector
            v.memset(junk.ap()[:], 0.0).then_inc(sems["j"], 1)
            last_bb_user = {0: None, 1: None}  # slot -> asm sem name

            def pos_block(p, b):
                dve.wait_ge(sems["exp"], p + 1)
                v.reciprocal(rs2.ap()[:, b : b + 1], sums2.ap()[:, b : b + 1])
                v.drain()
                if b in DIRECT:
                    v.tensor_scalar_mul(
                        attn_ball.ap()[b * A : (b + 1) * A, :],
                        probs[p % 2].ap()[:],
                        rs2.ap()[:, b : b + 1],
                    ).then_inc(sems["mulbf"], 1)
                else:
                    prev = last_bb_user[p % 2]
                    if prev is not None:
                        dve.wait_ge(sems[prev], 16)
                    v.tensor_scalar_mul(
                        attn_bb[p % 2].ap()[:],
                        probs[p % 2].ap()[:],
                        rs2.ap()[:, b : b + 1],
                    ).then_inc(sems["mulbf"], 1)
                    last_bb_user[p % 2] = f"asm{b}"

            def attnT_quarter(pi, ci):
                in_ = attn_ball.ap()[pi * 32 : (pi + 1) * 32, :].rearrange(
                    "p (t k c) -> p t k c", k=4, c=32
                )[:, :, ci, :]
                out = attnT.ap()[ci * 32 : (ci + 1) * 32, :, pi * 32 : pi * 32 + 32]
                v.transpose(out, in_).then_inc(sems["tt"], 1)

            for p in range(4):
                pos_block(p, B_SEQ[p])
            # lo-half transposes on DVE (off the critical chain)
            for b in (1, 2, 3):
                dve.wait_ge(sems[f"asm{b}"], 16)
            dve.wait_ge(sems["mulbf"], 4)  # retire b=0 direct write
            for ci in range(4):
                attnT_quarter(0, ci)
            for p in range(4, 8):
                pos_block(p, B_SEQ[p])
            # wdoc-lo psum -> bf16 wtmp (b=0..2 must precede the ttc copy:
            # PE's wdoc b=3 waits on cast 0 while the hi transposes sit
            # after wdoc-lo in the PE stream)
            for b in range(3):
                dve.wait_ge(sems["wd"], b + 1)
                v.tensor_copy(wtmp[b].ap()[:], ps_w[b % 3].ap()[:, :]).then_inc(
                    sems["wcp"], 1
                )
            # hi-half: copy the PE transposes out of PSUM
            dve.wait_ge(sems["tp"], ST)
            v.tensor_copy(attnT.ap()[:, :, 32:64], psT.ap()[:, :, :]).then_inc(
                sems["ttc"], 1
            )
            for b in range(3, BL):
                dve.wait_ge(sems["wd"], b + 1)
                v.tensor_copy(wtmp[b].ap()[:], ps_w[b % 3].ap()[:, :]).then_inc(
                    sems["wcp"], 1
                )
            # wallT out of PSUM
            dve.wait_ge(sems["wtp"], 1)
            v.tensor_copy(wallT.ap()[:], psWT.ap()[:]).then_inc(sems["wtc"], 1)
            # rep psum -> sbuf copies, even aspects
            for a in range(0, A, 2):
                dve.wait_ge(sems["rep"], a + 1)
                v.tensor_copy(
                    rep_all.ap()[:, a, :], ps_w[a % 3].ap()[:, :]
                ).then_inc(sems["repcv"], 1)

        @block.gpsimd
        def _(gp):
            nc.gpsimd.memset(zb.ap()[:], 0.0).then_inc(sems["z"], 1)

    nc.compile()
    return nc


def _get_nc():
    global _NC_CACHE
    if _NC_CACHE is None:
        _NC_CACHE = _build_nc()
    return _NC_CACHE


def make_in_maps(doc, aspE, aP):
    E = aspE.reshape(A, H, W)
    K = np.einsum("ahx,axw->ahw", aP, E)  # (A, H, W)
    kwid = np.zeros((128, 168), dtype=ml_dtypes.bfloat16)
    kwid[:, 0:128] = np.eye(128)
    kwid[:, 128 : 128 + W * A] = (
        K.transpose(1, 2, 0).reshape(H, W * A).astype(ml_dtypes.bfloat16)
    )
    apw = np.ascontiguousarray(aP.transpose(1, 0, 2).reshape(H, A * H)).astype(
        ml_dtypes.bfloat16
    )

    doc_bf = doc.astype(ml_dtypes.bfloat16)  # (B, S, H)
    in_maps = []
    for c in range(NCORES):
        dc = doc_bf[c * BL : (c + 1) * BL]  # (BL, S, H)
        m = {"kwid": kwid, "apw": apw}
        for i, half in enumerate(("A", "B")):
            dh = dc[i * 4 : (i + 1) * 4]
            # docN[p, b, t, h] = doc[b, t*128+p, h]  (batch order)
            m[f"docN{half}"] = np.ascontiguousarray(
                dh.reshape(4, ST, 128, H).transpose(2, 0, 1, 3)
            )
            # docT packed in process order B_SEQ
            dTc = np.zeros((128, 4, SP), dtype=ml_dtypes.bfloat16)
            for slot in range(4):
                blocal = B_SEQ[4 * i + slot] - 4 * i
                dTc[:, slot, PAD : PAD + S] = dh[blocal].transpose(1, 0)
            m[f"docT{half}"] = dTc
        in_maps.append(m)
    return in_maps


def kernel(batch_docIn, aspEmbed_weight, aspProj):
    global LAST_RESULT
    doc = np.asarray(batch_docIn, dtype=np.float32)
    aspE = np.asarray(aspEmbed_weight, dtype=np.float32)
    aP = np.asarray(aspProj, dtype=np.float32)
    in_maps = make_in_maps(doc, aspE, aP)

    nc = _get_nc()
    res = run_bass_kernel_spmd(
        nc, in_maps, core_ids=list(range(NCORES)), trace=TRACE
    )
    LAST_RESULT = res

    attn = np.empty((B, A, S), dtype=np.float32)
    rep = np.empty((B, A, H), dtype=np.float32)
    for c in range(NCORES):
        attn[c * BL : (c + 1) * BL] = (
            res.results[c]["attn_out"].astype(np.float32).reshape(BL, A, S)
        )
        rep[c * BL : (c + 1) * BL] = res.results[c]["rep_out"]
    return attn, rep
